# revision 2
# baseline (speedup 1.0000x reference)
"""RBF (Gaussian) kernel Gram matrix on 8 Trainium2 NeuronCores.

out[i, j] = exp(-gamma * ||x_i - y_j||^2),  x, y: [8192, 256] fp32.

Strategy (2x4 grid shard; v2 of the row-sharded baseline):
  - Cores arranged as 2 row-blocks (4096 rows of x) x 4 col-blocks (2048
    cols of y). vs. the 1D row shard this cuts per-core input DMA from
    6.5MB to 3.5MB and makes every output store a fully contiguous 512KB
    block of the core's [4096, 2048] fp16 stripe.
  - Per tile ([128 rows, 2048 cols]): PE does the k=256 fp16 GEMM
    (2 k-tiles x 4 PSUM banks, 215ns per 512-col matmul), ACT applies
    exp with the per-partition -g*||x||^2 bias straight out of PSUM
    (fp16 out), DVE multiplies by the per-column exp(-g*||y||^2) factor
    (fp16 tensor_tensor, 2x mode), DMA streams the stripe to DRAM.
  - The ACT queue carries ONLY ACTIVATEs plus a handful of early gating
    loads: in the baseline, 121 sem-wait + 7 DMA-issue instructions on
    the ACT queue added ~0.3us/tile to the pacing engine.
  - Gating set for the first tile is just 1.25MB (yt fully + first xt
    column chunks), so real matmuls start at ~3us (vs 17us baseline).
    Junk k=1 matmuls warm the PE HAM clock gate to 2.4GHz meanwhile.
  - POLY_TILES offloads whole tiles' exp from ACT (the pacing engine) to
    a degree-4 DVE polynomial path; each costs ~9us of otherwise-idle
    DVE time and removes 1.9us from ACT.
  - fp16 output halves HBM write traffic; host casts back to fp32.
"""

import numpy as np

GAMMA = 0.005
FULL_N = 8192
D = 256
N_CORES = 8
ROW_BLOCKS = 2
COL_BLOCKS = 4
M_SHARD = FULL_N // ROW_BLOCKS  # 4096 rows of x per core
N_SHARD = FULL_N // COL_BLOCKS  # 2048 cols of y per core
P = 128
M_TILES = M_SHARD // P  # 32
BANK = 512  # fp32 columns per PSUM bank (one matmul's max free dim)
N_BANKS = N_SHARD // BANK  # 4

# Tiles whose exp runs as a DVE polynomial instead of on the (pacing) ACT
# engine. P(u) is a degree-4 relative-minimax fit of e^u (squared later,
# u = g*x.y - g*||x||^2/2) in the nested form
#   c4*(u^4 + b3 u^3 + b2 u^2 + b1 u) + c0.
POLY_TILES = set()
PC4, PB3, PB2, PB1, PC0 = (
    0.021148114848613835,
    6.921948830443179,
    23.353569722985046,
    47.33351452617367,
    1.000357600651235,
)

_cache = {}


def _dedupe_ge_waits(nc):
    """Drop redundant sem-ge waits: engines execute their instructions in
    order and kernel-body semaphores only count up, so a >= wait whose
    threshold was already met by an earlier wait on the same engine is a
    no-op that still costs queue time (~84ns each once split). Skips any
    semaphore that is ever decremented (the barrier sems) and resets
    tracking at clear/reset instructions."""
    bad = set()
    for fn in nc.m.functions:
        for bb in fn.blocks:
            for inst in bb.instructions:
                si = inst.sync_info
                if si is not None:
                    for u in si.on_update:
                        if u.update_mode in ("sem-dec", "sem-sub-imm"):
                            bad.add(u.id)
    n_drop = 0
    for fn in nc.m.functions:
        for bb in fn.blocks:
            seen = {}
            for inst in bb.instructions:
                tn = type(inst).__name__
                if "Clear" in tn or "Reset" in tn:
                    seen = {}
                si = inst.sync_info
                if si is None or not si.on_wait:
                    continue
                kept = []
                for w in si.on_wait:
                    if (
                        w.wait_mode == "sem-ge-imm"
                        and w.id not in bad
                        and w.wait_reg is None
                    ):
                        key = (inst.engine, w.id)
                        if w.wait_value <= seen.get(key, -1):
                            n_drop += 1
                            continue
                        seen[key] = w.wait_value
                    kept.append(w)
                si.on_wait = kept
    return n_drop


def _split_sync_waits(nc, maxw=1):
    """walrus codegen rejects instructions carrying more than one sync wait
    (setupSyncWait fails even at 2). Tile can attach many (e.g. the tail
    drain waits on every semaphore; a matmul can wait on several DMA lanes).
    Hoist the excess onto wait-only EventSemaphore instructions inserted
    just before the offender on the same engine (engines execute their
    instructions in block order, so all waits still precede the op)."""
    import concourse.mybir as mybir

    def lim(inst):
        return maxw

    n_new = 0
    for fn in nc.m.functions:
        for bb in fn.blocks:
            insts = bb.instructions
            if not any(
                i.sync_info is not None and len(i.sync_info.on_wait) > lim(i)
                for i in insts
            ):
                continue
            new = []
            for inst in insts:
                maxw_i = lim(inst)
                si = inst.sync_info
                if si is not None and len(si.on_wait) > maxw_i:
                    waits = list(si.on_wait)
                    for i in range(0, len(waits) - maxw_i, maxw_i):
                        ev = mybir.InstEventSemaphore(
                            name=f"wsplit_{n_new}", ins=[], outs=[]
                        )
                        n_new += 1
                        ev.engine = inst.engine
                        ev.sync_info = mybir.SyncInfo(
                            on_wait=waits[i : i + maxw_i], on_update=[]
                        )
                        new.append(ev)
                    si.on_wait = waits[len(waits) - maxw_i :]
                new.append(inst)
            bb.instructions = new


def _build():
    import concourse.bass as bass
    import concourse.mybir as mybir
    import concourse.tile as tile

    f32 = mybir.dt.float32
    f16 = mybir.dt.float16
    alu = mybir.AluOpType
    nc = bass.Bass("TRN2", target_bir_lowering=False, debug=False)
    xt = nc.dram_tensor("xt", [D, M_SHARD], f16, kind="ExternalInput").ap()
    yt = nc.dram_tensor("yt", [D, N_SHARD], f16, kind="ExternalInput").ap()
    # cols 0..M_TILES-1: -g*||x||^2 (ACT exp bias); cols M_TILES..: half of
    # that (bias for the DVE polynomial-exp path, which works on s/2).
    x2 = nc.dram_tensor("x2", [P, 2 * M_TILES], f32, kind="ExternalInput").ap()
    eyr = nc.dram_tensor("eyr", [1, N_SHARD], f16, kind="ExternalInput").ap()
    out = nc.dram_tensor("out", [M_SHARD, N_SHARD], f16, kind="ExternalOutput").ap()

    with tile.TileContext(nc) as tc:
        with (
            tc.tile_pool(name="const", bufs=1) as cpool,
            tc.tile_pool(name="actp", bufs=6) as apool,
            tc.tile_pool(name="outp", bufs=6) as opool,
            tc.tile_pool(name="poly", bufs=2) as spool,
            tc.tile_pool(name="psum", bufs=2, space="PSUM") as ppool,
        ):
            ones = cpool.tile([1, P], f16, tag="ones")
            nc.any.memset(ones, 1.0)
            wrow = cpool.tile([1, BANK], f16, tag="wrow")
            nc.any.memset(wrow, 1.0)
            xt0 = cpool.tile([P, M_SHARD], f16, tag="xt0")
            xt1 = cpool.tile([P, M_SHARD], f16, tag="xt1")
            x2sb = cpool.tile([P, 2 * M_TILES], f32, tag="x2")
            yt0 = cpool.tile([P, N_SHARD], f16, tag="yt0")
            yt1 = cpool.tile([P, N_SHARD], f16, tag="yt1")
            ey = cpool.tile([P, N_SHARD], f16, tag="ey")

            # Gating loads, most-urgent first, alternated over the two
            # HWDGE queues (SP + ACT; ACT is idle until ~4us so a few
            # issues there are free). The first matmul needs only
            # xt0[:, 0:128] and yt0 bank 0; later banks land just in time
            # behind it. 128KB chunks (one bank / 512 xt cols) keep many
            # transfers in flight - a single dma_start only reaches
            # ~60GB/s.
            XCH = 512  # xt gating chunk columns
            gating = []
            gating.append((yt0[:, 0:BANK], yt[0:P, 0:BANK]))
            gating.append((xt0[:, 0:XCH], xt[0:P, 0:XCH]))
            gating.append((yt0[:, BANK : 2 * BANK], yt[0:P, BANK : 2 * BANK]))
            gating.append((xt1[:, 0:XCH], xt[P : 2 * P, 0:XCH]))
            gating.append((yt0[:, 2 * BANK : 3 * BANK], yt[0:P, 2 * BANK : 3 * BANK]))
            gating.append((yt1[:, 0:BANK], yt[P : 2 * P, 0:BANK]))
            gating.append((yt0[:, 3 * BANK : 4 * BANK], yt[0:P, 3 * BANK : 4 * BANK]))
            gating.append((yt1[:, BANK : 2 * BANK], yt[P : 2 * P, BANK : 2 * BANK]))
            gating.append((yt1[:, 2 * BANK : 3 * BANK], yt[P : 2 * P, 2 * BANK : 3 * BANK]))
            gating.append((yt1[:, 3 * BANK : 4 * BANK], yt[P : 2 * P, 3 * BANK : 4 * BANK]))
            qs = [nc.sync, nc.scalar]
            for i, (dst, src) in enumerate(gating):
                qs[i % 2].dma_start(out=dst, in_=src)
            nc.sync.dma_start(out=x2sb, in_=x2)
            # Preload the ACT exp table set (~1.3us) off the critical path:
            # the first real ACTIVATE would otherwise trigger it lazily.
            tldw = cpool.tile([1, 1], f16, tag="tldw")
            nc.scalar.activation(
                tldw, ones[:, 0:1], mybir.ActivationFunctionType.Exp
            )
            # Bulk loads on the (otherwise idle) GPSIMD software-DGE queue:
            # the per-column factor ey arrives via a broadcast
            # (partition-stride-0) DRAM source re-reading a hot 4KB row;
            # the xt tail chunks are needed a tile-time (1.9us) per 512
            # cols, far behind the Q7 issue rate.
            nc.gpsimd.dma_start(
                out=ey, in_=eyr.to_broadcast((P, N_SHARD))
            )
            for c in range(XCH, M_SHARD, XCH):
                nc.gpsimd.dma_start(out=xt0[:, c : c + XCH], in_=xt[0:P, c : c + XCH])
                nc.gpsimd.dma_start(
                    out=xt1[:, c : c + XCH], in_=xt[P : 2 * P, c : c + XCH]
                )

            # HAM warmup: only MATMUL activity flips the PE clock gate from
            # 1.2GHz to 2.4GHz (takes a ~3.4us-busy window). These junk k=1
            # matmuls (never read; WAW into a rotating psum tile is safe)
            # depend only on the memsets, so the PE is busy from ~0.5us and
            # hands off to the first real matmuls as their inputs land.
            psw = ppool.tile([P, N_SHARD], f32, tag="ps")
            for w in range(7):
                nc.tensor.matmul(
                    psw[:, 0:BANK], ones, wrow, start=True, stop=True
                )
            for w in range(4):
                nc.tensor.matmul(
                    psw[:, 0:P], ones, wrow[:, 0:P], start=True, stop=True
                )

            # Main loop over the 32 row-tiles of the [4096, 2048] stripe.
            # The last tile's act+mult+store run in 1024-col halves to
            # shorten the drain tail.
            for t in range(M_TILES):
                last = t == M_TILES - 1
                msl = slice(t * P, (t + 1) * P)
                ps = ppool.tile([P, N_SHARD], f32, tag="ps")
                for d, (xtd, ytd) in enumerate(((xt0, yt0), (xt1, yt1))):
                    for b in range(N_BANKS):
                        bsl = slice(b * BANK, (b + 1) * BANK)
                        nc.tensor.matmul(
                            ps[:, bsl], xtd[:, msl], ytd[:, bsl],
                            start=(d == 0), stop=(d == 1),
                        )
                if t in POLY_TILES:
                    # Polynomial-exp path: u = g*ps - g*||x||^2/2, then
                    # e^(2u) ~= (c4*(u^4+b3 u^3+b2 u^2+b1 u)+c0)^2, then
                    # the per-column ey factor. ~9us of DVE per tile vs
                    # 1.9us of ACT; the drain (first op) holds PSUM for
                    # 2.3us (fp32 1x mode).
                    x2h = x2sb[:, M_TILES + t : M_TILES + t + 1]
                    u = spool.tile([P, N_SHARD], f16, tag="pu")
                    nc.vector.tensor_scalar(
                        u, ps, GAMMA, x2h, op0=alu.mult, op1=alu.add
                    )
                    h1 = spool.tile([P, N_SHARD], f16, tag="ph1")
                    nc.vector.scalar_tensor_tensor(
                        h1, u, PB3, u, op0=alu.add, op1=alu.mult
                    )
                    h2 = spool.tile([P, N_SHARD], f16, tag="ph2")
                    nc.vector.scalar_tensor_tensor(
                        h2, h1, PB2, u, op0=alu.add, op1=alu.mult
                    )
                    h3 = spool.tile([P, N_SHARD], f16, tag="ph1")
                    nc.vector.scalar_tensor_tensor(
                        h3, h2, PB1, u, op0=alu.add, op1=alu.mult
                    )
                    pp = spool.tile([P, N_SHARD], f16, tag="ph2")
                    nc.vector.tensor_scalar(
                        pp, h3, PC4, PC0, op0=alu.mult, op1=alu.add
                    )
                    sq = spool.tile([P, N_SHARD], f16, tag="pu")
                    nc.vector.tensor_mul(sq, pp, pp)
                    ot = opool.tile([P, N_SHARD], f16, tag="ot")
                    nc.vector.tensor_mul(ot, sq, ey)
                    nc.gpsimd.dma_start(out=out[msl, :], in_=ot)
                    continue
                at = apool.tile([P, N_SHARD], f16, tag="at")
                ot = opool.tile([P, N_SHARD], f16, tag="ot")
                if last:
                    pieces = [(0, N_SHARD // 2), (N_SHARD // 2, N_SHARD)]
                else:
                    pieces = [(0, N_SHARD)]
                for pi, (h0, h1) in enumerate(pieces):
                    hsl = slice(h0, h1)
                    # exp(2g*(x.y) - g*||x||^2): bias is per-partition,
                    # free on the ACT datapath.
                    nc.scalar.activation(
                        at[:, hsl], ps[:, hsl],
                        mybir.ActivationFunctionType.Exp,
                        bias=x2sb[:, t : t + 1], scale=2.0 * GAMMA,
                    )
                    nc.vector.tensor_mul(ot[:, hsl], at[:, hsl], ey[:, hsl])
                    # Stores alternate between the SP HWDGE queue and the
                    # GPSIMD SWDGE queue so neither serializes; the output
                    # stripe layout makes each store one contiguous 512KB
                    # DRAM block.
                    if last:
                        q = qs[pi % 2]
                    else:
                        q = nc.sync if t % 2 == 0 else nc.gpsimd
                    q.dma_start(out=out[msl, hsl], in_=ot[:, hsl])

    _dedupe_ge_waits(nc)
    _split_sync_waits(nc, maxw=1)
    return nc


def kernel(x: np.ndarray, y: np.ndarray) -> np.ndarray:
    from concourse import bass_utils

    x = np.asarray(x, dtype=np.float32)
    y = np.asarray(y, dtype=np.float32)

    if "nc" not in _cache:
        _cache["nc"] = _build()
    nc = _cache["nc"]

    xt_full = x.T.astype(np.float16)  # [256, 8192]
    yt_full = np.ascontiguousarray(y.T.astype(np.float16))  # [256, 8192]
    x2 = np.sum(x.astype(np.float64) * x.astype(np.float64), axis=1)  # [8192]
    y2 = np.sum(y.astype(np.float64) * y.astype(np.float64), axis=1)
    ey_full = np.exp(-GAMMA * y2).astype(np.float16)  # [8192]

    in_maps = []
    for c in range(N_CORES):
        r = c // COL_BLOCKS
        q = c % COL_BLOCKS
        rows = slice(r * M_SHARD, (r + 1) * M_SHARD)
        cols = slice(q * N_SHARD, (q + 1) * N_SHARD)
        x2c = (-GAMMA * x2[rows]).astype(np.float32)
        x2t = x2c.reshape(M_TILES, P).T  # [P, M_TILES]
        in_maps.append(
            {
                "xt": np.ascontiguousarray(xt_full[:, rows]),
                "yt": np.ascontiguousarray(yt_full[:, cols]),
                "x2": np.ascontiguousarray(
                    np.concatenate([x2t, 0.5 * x2t], axis=1)
                ),
                "eyr": ey_full[cols].reshape(1, N_SHARD),
            }
        )

    res = bass_utils.run_bass_kernel_spmd(
        nc, in_maps, core_ids=list(range(N_CORES))
    )
    _cache["last_result"] = res
    full = np.empty((FULL_N, FULL_N), dtype=np.float32)
    for c in range(N_CORES):
        r = c // COL_BLOCKS
        q = c % COL_BLOCKS
        full[
            r * M_SHARD : (r + 1) * M_SHARD, q * N_SHARD : (q + 1) * N_SHARD
        ] = res.results[c]["out"].astype(np.float32)
    return full
